# revision 1
# baseline (speedup 1.0000x reference)
"""Trainium2 Bass kernel for nn_ModelNew_25056839205117 (SK-style branch fuse).

Reference computation (B=32, C=256, H=W=56, K=4):
    u     = x0+x1+x2+x3                  [B,C,H,W]
    s     = mean_{H,W}(u)                [B,C]
    w     = softmax_K(attn_logits)       [K,B,C]
    out   = sum_k w[k]*x_k               [B,C,H,W]
    u_sum = sum_{H,W}(u)                 [B,C]
    returns (out, s, u_sum)

Strategy: data-parallel over batch across 8 NeuronCores (4 batches/core).
Per core, 8 iterations over (b_local, channel-half): tiles are
[128 channels (partitions) x 3136 HW (free)], fully DMA-contiguous.
  - DVE: acc = w0*x0, then 3x fused scalar_tensor_tensor acc = wk*xk + acc
  - ACT: 4x identity-copy with fused accum_out -> per-branch HW-sums r_k
    (u_sum = sum_k r_k, exact unweighted reduction)
  - softmax over K=4 done on-device on a tiny [128,16] logits tile
    (host only re-lays-out attn_logits to [C, B_local, K] per core).
Memory-bound: ~61 MiB HBM traffic/core -> ~180 us roofline at 358 GB/s.
"""

import numpy as np

B, C, H, W, K = 32, 256, 56, 56, 4
HW = H * W          # 3136
NCORES = 8
BLOC = B // NCORES  # 4
NCT = C // 128      # 2 channel-tiles per batch

_cache = {}


def _build():
    """Build + compile the SPMD Bass program (once per process)."""
    if "nc" in _cache:
        return _cache["nc"]

    import concourse.bacc as bacc
    import concourse.mybir as mybir
    from concourse import tile

    f32 = mybir.dt.float32
    AF = mybir.ActivationFunctionType
    ALU = mybir.AluOpType
    AX = mybir.AxisListType

    nc = bacc.Bacc("TRN2", target_bir_lowering=False, debug=False,
                   num_devices=NCORES)

    x_dr = [
        nc.dram_tensor(f"x{k}", [BLOC, C, HW], f32, kind="ExternalInput").ap()
        for k in range(K)
    ]
    # logits, host-relaid-out to [C, B_local*K] with K innermost
    wl_dr = nc.dram_tensor("wl", [C, BLOC * K], f32, kind="ExternalInput").ap()
    out_dr = nc.dram_tensor("out", [BLOC, C, HW], f32, kind="ExternalOutput").ap()
    us_dr = nc.dram_tensor("usum", [BLOC, C], f32, kind="ExternalOutput").ap()
    s_dr = nc.dram_tensor("s", [BLOC, C], f32, kind="ExternalOutput").ap()

    with tile.TileContext(nc) as tc:
        with (
            tc.tile_pool(name="xin", bufs=2) as xin_pool,
            tc.tile_pool(name="accp", bufs=3) as acc_pool,
            tc.tile_pool(name="scrapp", bufs=1) as scrap_pool,
            tc.tile_pool(name="small", bufs=1) as small_pool,
        ):
            # ---- softmax over K for the fuse weights, per channel-tile ----
            wts = []
            for ct in range(NCT):
                lt = small_pool.tile([128, BLOC * K], f32, tag=f"lt{ct}")
                nc.sync.dma_start(out=lt[:], in_=wl_dr[ct * 128:(ct + 1) * 128, :])
                et = small_pool.tile([128, BLOC * K], f32, tag=f"et{ct}")
                nc.scalar.activation(et[:], lt[:], AF.Exp)
                dt_ = small_pool.tile([128, BLOC], f32, tag=f"dt{ct}")
                nc.vector.tensor_reduce(
                    out=dt_[:],
                    in_=et[:].rearrange("p (b k) -> p b k", k=K),
                    axis=AX.X,
                    op=ALU.add,
                )
                rt = small_pool.tile([128, BLOC], f32, tag=f"rt{ct}")
                nc.vector.reciprocal(rt[:], dt_[:])
                wt = small_pool.tile([128, BLOC * K], f32, tag=f"wt{ct}")
                for b in range(BLOC):
                    nc.vector.tensor_scalar_mul(
                        wt[:, K * b:K * (b + 1)],
                        et[:, K * b:K * (b + 1)],
                        rt[:, b:b + 1],
                    )
                wts.append(wt)

            # per-(iter,k) branch HW-sums land here: col = (b*NCT+ct)*K + k
            r_all = small_pool.tile([128, BLOC * NCT * K], f32, tag="r_all")
            scrap = scrap_pool.tile([128, HW], f32, tag="scrap")

            # ---- main streaming loop ----
            for b in range(BLOC):
                for ct in range(NCT):
                    col = b * NCT + ct
                    cs = slice(ct * 128, (ct + 1) * 128)
                    xts = []
                    for k in range(K):
                        xt = xin_pool.tile([128, HW], f32, tag=f"x{k}")
                        nc.sync.dma_start(out=xt[:], in_=x_dr[k][b, cs, :])
                        xts.append(xt)
                    wt = wts[ct]
                    acc = acc_pool.tile([128, HW], f32, tag="acc")
                    nc.vector.tensor_scalar_mul(
                        acc[:], xts[0][:], wt[:, K * b:K * b + 1])
                    for k in range(1, K):
                        nc.vector.scalar_tensor_tensor(
                            out=acc[:],
                            in0=xts[k][:],
                            scalar=wt[:, K * b + k:K * b + k + 1],
                            in1=acc[:],
                            op0=ALU.mult,
                            op1=ALU.add,
                        )
                    for k in range(K):
                        nc.scalar.activation(
                            scrap[:], xts[k][:], AF.Copy,
                            accum_out=r_all[:, col * K + k:col * K + k + 1],
                        )
                    nc.sync.dma_start(out=out_dr[b, cs, :], in_=acc[:])

            # ---- tail: u_sum and s from the accumulated branch sums ----
            us_t = small_pool.tile([128, BLOC * NCT], f32, tag="us_t")
            nc.vector.tensor_reduce(
                out=us_t[:],
                in_=r_all[:].rearrange("p (c k) -> p c k", k=K),
                axis=AX.X,
                op=ALU.add,
            )
            s_t = small_pool.tile([128, BLOC * NCT], f32, tag="s_t")
            nc.vector.tensor_scalar_mul(s_t[:], us_t[:], 1.0 / float(HW))
            nc.sync.dma_start(
                out=us_dr.rearrange("b (t p) -> p b t", p=128),
                in_=us_t[:].rearrange("p (b t) -> p b t", t=NCT),
            )
            nc.sync.dma_start(
                out=s_dr.rearrange("b (t p) -> p b t", p=128),
                in_=s_t[:].rearrange("p (b t) -> p b t", t=NCT),
            )

    nc.compile()
    _cache["nc"] = nc
    return nc


def make_in_maps(x0, x1, x2, x3, attn_logits):
    """Shard full inputs into per-core input maps (host-side, views/cheap)."""
    xs = [np.ascontiguousarray(x, dtype=np.float32).reshape(B, C, HW)
          for x in (x0, x1, x2, x3)]
    att = np.ascontiguousarray(attn_logits, dtype=np.float32)
    in_maps = []
    for c in range(NCORES):
        b0 = BLOC * c
        # [K, BLOC, C] -> [C, BLOC, K] -> [C, BLOC*K]
        wl = np.ascontiguousarray(
            att[:, b0:b0 + BLOC, :].transpose(2, 1, 0)).reshape(C, BLOC * K)
        in_maps.append({
            "x0": xs[0][b0:b0 + BLOC],
            "x1": xs[1][b0:b0 + BLOC],
            "x2": xs[2][b0:b0 + BLOC],
            "x3": xs[3][b0:b0 + BLOC],
            "wl": wl,
        })
    return in_maps


def assemble(results):
    """Gather per-core result dicts into full (out, s, u_sum)."""
    out = np.concatenate([r["out"] for r in results], axis=0)
    out = out.reshape(B, C, H, W)
    s = np.concatenate([r["s"] for r in results], axis=0)
    usum = np.concatenate([r["usum"] for r in results], axis=0)
    return out, s, usum


def kernel(x0, x1, x2, x3, attn_logits):
    from concourse.bass_utils import run_bass_kernel_spmd

    nc = _build()
    in_maps = make_in_maps(x0, x1, x2, x3, attn_logits)
    res = run_bass_kernel_spmd(nc, in_maps, core_ids=list(range(NCORES)))
    return assemble(res.results)


# revision 42
# speedup vs baseline: 838.4863x; 838.4863x over previous
"""Trainium2 Bass kernel for nn_ModelNew_25056839205117 (SK-style branch fuse).

Reference computation (B=32, C=256, H=W=56, K=4):
    u     = x0+x1+x2+x3                  [B,C,H,W]
    s     = mean_{H,W}(u)                [B,C]
    w     = softmax_K(attn_logits)       [K,B,C]
    out   = sum_k w[k]*x_k               [B,C,H,W]
    u_sum = sum_{H,W}(u)                 [B,C]
    returns (out, s, u_sum)

Strategy: data-parallel over batch across 8 NeuronCores (4 batches/core).
Per core, 8 iterations over (b_local, channel-half): tiles are
[128 channels (partitions) x 3136 HW (free)], fully DMA-contiguous.
  - DVE: acc = w0*x0, then 3x fused scalar_tensor_tensor acc = wk*xk + acc
  - ACT: 4x identity-copy with fused accum_out -> per-branch HW-sums r_k
    (u_sum = sum_k r_k, exact unweighted reduction)
  - softmax over K=4 done on-device on a tiny [128,16] logits tile
    (host only re-lays-out attn_logits to [C, B_local, K] per core).
Memory-bound: ~61 MiB HBM traffic/core -> ~180 us roofline at 358 GB/s.
"""

import numpy as np

B, C, H, W, K = 32, 256, 56, 56, 4
HW = H * W          # 3136
NCORES = 8
BLOC = B // NCORES  # 4
NCT = C // 128      # 2 channel-tiles per batch

_cache = {}


NCHUNK = 1      # HW-plane split per iteration (1 => free dim 3136)
XIN_BUFS = 2
ACC_BUFS = 3


def _build(reps=1, loop=0, nchunk=None, xin_bufs=None, acc_bufs=None,
           dma_only=False, store_eng="sync", split_in=True,
           store_alt=True, rr=False, act_off=False, stack_x=False,
           ld_eng="alt", stack_pairs=False):
    """Build + compile the SPMD Bass program (once per process).

    reps>1 unrolls the whole computation that many times (same I/O);
    loop=R>0 instead wraps one rep in a device-side For_i loop run R
    times. Both are used only for timing measurements, never grading.
    dma_only drops all compute (DMA-floor probe); store_eng picks the
    engine issuing the output DMA.
    """
    # default config (graded path): loads k0,k1 on the SP HWDGE ring and
    # k2,k3 on the ACT ring, stores alternating between rings — measured
    # fastest without requiring host-side restacking.
    nchunk = NCHUNK if nchunk is None else nchunk
    xin_bufs = XIN_BUFS if xin_bufs is None else xin_bufs
    acc_bufs = ACC_BUFS if acc_bufs is None else acc_bufs
    key = ("nc", reps, loop, nchunk, xin_bufs, acc_bufs, dma_only, store_eng,
           split_in, store_alt, rr, act_off, stack_x, ld_eng, stack_pairs)
    if key in _cache:
        return _cache[key]

    import concourse.bacc as bacc
    import concourse.mybir as mybir
    from concourse import tile

    f32 = mybir.dt.float32
    AF = mybir.ActivationFunctionType
    ALU = mybir.AluOpType
    AX = mybir.AxisListType

    nc = bacc.Bacc("TRN2", target_bir_lowering=False, debug=False,
                   num_devices=NCORES)

    if stack_x or stack_pairs:
        xs_dr = nc.dram_tensor("xs", [K, BLOC, C, HW], f32,
                               kind="ExternalInput").ap()
        x_dr = None
    else:
        x_dr = [
            nc.dram_tensor(f"x{k}", [BLOC, C, HW], f32,
                           kind="ExternalInput").ap()
            for k in range(K)
        ]
    # logits, host-relaid-out to [C, B_local*K] with K innermost
    wl_dr = nc.dram_tensor("wl", [C, BLOC * K], f32, kind="ExternalInput").ap()
    out_dr = nc.dram_tensor("out", [BLOC, C, HW], f32, kind="ExternalOutput").ap()
    us_dr = nc.dram_tensor("usum", [BLOC, C], f32, kind="ExternalOutput").ap()
    s_dr = nc.dram_tensor("s", [BLOC, C], f32, kind="ExternalOutput").ap()

    def emit_once(tc, xin_pool, acc_pool, scrap_pool, small_pool):
        # ---- softmax over K for the fuse weights, per channel-tile ----
        wts = []
        for ct in range(NCT):
            lt = small_pool.tile([128, BLOC * K], f32, tag=f"lt{ct}")
            nc.sync.dma_start(out=lt[:], in_=wl_dr[ct * 128:(ct + 1) * 128, :])
            et = small_pool.tile([128, BLOC * K], f32, tag=f"et{ct}")
            nc.scalar.activation(et[:], lt[:], AF.Exp)
            dt_ = small_pool.tile([128, BLOC], f32, tag=f"dt{ct}")
            nc.vector.tensor_reduce(
                out=dt_[:],
                in_=et[:].rearrange("p (b k) -> p b k", k=K),
                axis=AX.X,
                op=ALU.add,
            )
            rt = small_pool.tile([128, BLOC], f32, tag=f"rt{ct}")
            nc.vector.reciprocal(rt[:], dt_[:])
            wt = small_pool.tile([128, BLOC * K], f32, tag=f"wt{ct}")
            for b in range(BLOC):
                nc.vector.tensor_scalar_mul(
                    wt[:, K * b:K * (b + 1)],
                    et[:, K * b:K * (b + 1)],
                    rt[:, b:b + 1],
                )
            wts.append(wt)

        # per-(iter,chunk,k) branch HW-sums: col*(nchunk*K) + ch*K + k
        if not dma_only:
            r_all = small_pool.tile([128, BLOC * NCT * nchunk * K], f32,
                                    tag="r_all")
            if not act_off:
                scrap = scrap_pool.tile([128, HW // nchunk], f32,
                                        tag="scrap")

        # ---- main streaming loop ----
        fw = HW // nchunk  # free-dim width per chunk
        for b in range(BLOC):
            for ct in range(NCT):
                col = b * NCT + ct
                cs = slice(ct * 128, (ct + 1) * 128)
                for ch in range(nchunk):
                    fs = slice(ch * fw, (ch + 1) * fw)
                    xts = []
                    if stack_pairs:
                        for half in range(2):
                            xp = xin_pool.tile([128, 2, fw], f32,
                                               tag=f"xp{half}")
                            if ld_eng == "alt":
                                ld = nc.scalar if half else nc.sync
                            else:
                                ld = getattr(nc, ld_eng)
                            ld.dma_start(
                                out=xp[:],
                                in_=xs_dr[2 * half:2 * half + 2, b, cs, fs]
                                .rearrange("k p f -> p k f"),
                            )
                            xts.extend([xp[:, 0, :], xp[:, 1, :]])
                    elif stack_x:
                        xt_all = xin_pool.tile([128, K, fw], f32, tag="xall")
                        if ld_eng == "alt":
                            ld = nc.scalar if (col * nchunk + ch) % 2 \
                                else nc.sync
                        else:
                            ld = getattr(nc, ld_eng)
                        ld.dma_start(
                            out=xt_all[:],
                            in_=xs_dr[:, b, cs, fs].rearrange(
                                "k p f -> p k f"),
                        )
                        xts = [xt_all[:, k, :] for k in range(K)]
                    else:
                        for k in range(K):
                            xt = xin_pool.tile([128, fw], f32, tag=f"x{k}")
                            if rr:
                                ld = nc.scalar if (col * nchunk + ch + k) % 2 \
                                    else nc.sync
                            else:
                                ld = nc.scalar if (split_in and k >= 2) \
                                    else nc.sync
                            ld.dma_start(out=xt[:], in_=x_dr[k][b, cs, fs])
                            xts.append(xt)
                    wt = wts[ct]
                    if dma_only:
                        store = getattr(nc, store_eng)
                        store.dma_start(out=out_dr[b, cs, fs], in_=xts[0][:])
                        continue
                    acc = acc_pool.tile([128, fw], f32, tag="acc")
                    if not dma_only:
                        nc.vector.tensor_scalar_mul(
                            acc[:], xts[0][:], wt[:, K * b:K * b + 1])
                        for k in range(1, K):
                            nc.vector.scalar_tensor_tensor(
                                out=acc[:],
                                in0=xts[k][:],
                                scalar=wt[:, K * b + k:K * b + k + 1],
                                in1=acc[:],
                                op0=ALU.mult,
                                op1=ALU.add,
                            )
                        for k in range(K):
                            # per-branch free-dim reduce -> r_k
                            j = (col * nchunk + ch) * K + k
                            if act_off:
                                nc.vector.tensor_reduce(
                                    out=r_all[:, j:j + 1],
                                    in_=xts[k][:],
                                    axis=AX.X,
                                    op=ALU.add,
                                )
                            else:
                                nc.scalar.activation(
                                    scrap[:], xts[k][:], AF.Copy,
                                    accum_out=r_all[:, j:j + 1],
                                )
                    if store_alt:
                        store = nc.sync if (col * nchunk + ch) % 2 else nc.scalar
                    else:
                        store = getattr(nc, store_eng)
                    store.dma_start(out=out_dr[b, cs, fs], in_=acc[:])

        # ---- tail: u_sum and s from the accumulated branch sums ----
        if dma_only:
            return
        us_t = small_pool.tile([128, BLOC * NCT], f32, tag="us_t")
        nc.vector.tensor_reduce(
            out=us_t[:],
            in_=r_all[:].rearrange("p (c j) -> p c j", j=nchunk * K),
            axis=AX.X,
            op=ALU.add,
        )
        s_t = small_pool.tile([128, BLOC * NCT], f32, tag="s_t")
        nc.vector.tensor_scalar_mul(s_t[:], us_t[:], 1.0 / float(HW))
        nc.sync.dma_start(
            out=us_dr.rearrange("b (t p) -> p b t", p=128),
            in_=us_t[:].rearrange("p (b t) -> p b t", t=NCT),
        )
        nc.sync.dma_start(
            out=s_dr.rearrange("b (t p) -> p b t", p=128),
            in_=s_t[:].rearrange("p (b t) -> p b t", t=NCT),
        )

    with tile.TileContext(nc) as tc:
        with (
            tc.tile_pool(name="xin", bufs=xin_bufs) as xin_pool,
            tc.tile_pool(name="accp", bufs=acc_bufs) as acc_pool,
            tc.tile_pool(name="scrapp", bufs=1) as scrap_pool,
            tc.tile_pool(name="small", bufs=2) as small_pool,
        ):
            if loop:
                with tc.For_i(0, loop, 1):
                    emit_once(tc, xin_pool, acc_pool, scrap_pool, small_pool)
            else:
                for _rep in range(reps):
                    emit_once(tc, xin_pool, acc_pool, scrap_pool, small_pool)

    nc.compile()
    _cache[key] = nc
    return nc


def make_in_maps(x0, x1, x2, x3, attn_logits, stack_x=False):
    """Shard full inputs into per-core input maps (host-side, views/cheap)."""
    xs = [np.ascontiguousarray(x, dtype=np.float32).reshape(B, C, HW)
          for x in (x0, x1, x2, x3)]
    att = np.ascontiguousarray(attn_logits, dtype=np.float32)
    in_maps = []
    for c in range(NCORES):
        b0 = BLOC * c
        # [K, BLOC, C] -> [C, BLOC, K] -> [C, BLOC*K]
        wl = np.ascontiguousarray(
            att[:, b0:b0 + BLOC, :].transpose(2, 1, 0)).reshape(C, BLOC * K)
        if stack_x:
            in_maps.append({
                "xs": np.stack([x[b0:b0 + BLOC] for x in xs]),
                "wl": wl,
            })
        else:
            in_maps.append({
                "x0": xs[0][b0:b0 + BLOC],
                "x1": xs[1][b0:b0 + BLOC],
                "x2": xs[2][b0:b0 + BLOC],
                "x3": xs[3][b0:b0 + BLOC],
                "wl": wl,
            })
    return in_maps


def assemble(results):
    """Gather per-core result dicts into full (out, s, u_sum)."""
    out = np.concatenate([r["out"] for r in results], axis=0)
    out = out.reshape(B, C, H, W)
    s = np.concatenate([r["s"] for r in results], axis=0)
    usum = np.concatenate([r["usum"] for r in results], axis=0)
    return out, s, usum


def kernel(x0, x1, x2, x3, attn_logits):
    from concourse.bass_utils import run_bass_kernel_spmd

    nc = _build()
    in_maps = make_in_maps(x0, x1, x2, x3, attn_logits)
    res = run_bass_kernel_spmd(nc, in_maps, core_ids=list(range(NCORES)))
    return assemble(res.results)
